# revision 23
# baseline (speedup 1.0000x reference)
"""Multi-head attention (no softmax) Trainium2 Bass kernel, 8-core SPMD.

Reference computes out = ((X Wq + bq)(X Wk + bk)^T / 8)(X Wv + bv) Wo + bo
per head.  Since there is no softmax the per-head attention is linear:
    (Q_h K_h^T) V_h = Q_h (K_h^T V_h)
which collapses the 2048x2048 score matrix to a 64x64 per-head matrix.
Further folding (K_h^T V_h / 8) Wo_h into a data-dependent weight
W~ = stack_h(M_h Wo_h) makes the whole core computation:
    Q = X Wq_s,  KV = X [Wk_s|Wv_s],  MT_h = V_h^T K_h,
    W~_h = (MT_h^T / 8) Wo_h,  P = Q W~        (P is a partial of out)

Sharding: core c -> batch b = c // 4, head-group g = c % 4 (4 of 16 heads,
256 of 1024 projection columns).  No cross-device comms; the 4 partials per
batch are summed on the host (+ bo).

Matmuls run in float32r (tf32: 10-bit mantissa operands, fp32 accumulate)
which streams at 1 cycle/row on the PE vs 4 for fp32.  Inputs are
pre-rounded (RNE) to tf32 on the host; intermediates round on the DVE
copy-back write.  Set USE_F32R = False for full-fp32 matmuls.

Schedule: dummy zero-matmuls warm the PE clock gate while the first DMAs
land; K|V projections chase the x^T token-block loads; MT/W~ form the one
unavoidable sync point; then Q^T and P = Q W~ interleave per token block so
the output DMA overlaps the remaining compute.  Measured ~93us HW time per
8-core launch (PE-dense; ~59us pure matmul streaming + fixed pro/epilogue).
"""

import numpy as np

import concourse.mybir as mybir
import concourse.tile as tile
from concourse import bacc
from concourse.bass_utils import run_bass_kernel_spmd

F32 = mybir.dt.float32
F32R = mybir.dt.float32r

USE_F32R = True
MM_DT = F32R if USE_F32R else F32

B, L, DM = 2, 2048, 1024
QD = 256                 # per-core projection width (4 heads x 64)
HPC, HDIM = 4, 64
NCORES = 8
SCALE = 0.125            # 1 / sqrt(64)

DM_C = DM // 128         # 8 dmodel chunks
T_N = L // 128           # 16 token chunks (partition-sized)
T_F = L // 512           # 4 token chunks (free-dim sized)
QD_C = QD // 128         # 2 head-dim chunks / head-pairs
OUT_F = DM // 512        # 2 output free chunks


def build_program():
    nc = bacc.Bacc("TRN2", target_bir_lowering=False, debug=False)

    xT = nc.dram_tensor("xT", [DM, L], MM_DT, kind="ExternalInput")
    wq = nc.dram_tensor("wq", [DM, QD], MM_DT, kind="ExternalInput")
    wkv = nc.dram_tensor("wkv", [DM, 2 * QD], MM_DT, kind="ExternalInput")
    wo = nc.dram_tensor("wo", [QD, DM], MM_DT, kind="ExternalInput")
    bqt = nc.dram_tensor("bqt", [128, QD_C], F32, kind="ExternalInput")
    bkd = nc.dram_tensor("bkd", [1, QD], F32, kind="ExternalInput")
    bvd = nc.dram_tensor("bvd", [1, QD], F32, kind="ExternalInput")
    pout = nc.dram_tensor("pout", [L, DM], F32, kind="ExternalOutput")

    with tile.TileContext(nc) as tc:
        with (
            tc.tile_pool(name="persist", bufs=1) as pers,
            tc.tile_pool(name="pstage", bufs=4) as pstage,
            tc.tile_pool(name="psum", bufs=8, space="PSUM") as ps,
        ):
            # -------- PE warm-up: dummy matmuls with no DMA deps -----------
            # The HAM clock gate keeps an idle PE at 1.2 GHz; ~3.5us of
            # sustained activity moves it to 2.4 GHz.  These zero-matmuls run
            # while the DMA descriptors are still being generated, so the
            # first real matmul executes at full rate.
            warm_z = pers.tile([128, 512], MM_DT, tag="warmz")
            nc.vector.memzero(warm_z[:])
            warm_out = pers.tile([128, 512], F32, tag="warmout")
            pswarm = ps.tile([128, 512], F32, tag="ps", name="pswarm")
            for i in range(11):
                nc.tensor.matmul(
                    pswarm[:], lhsT=warm_z[:, 0:128], rhs=warm_z[:],
                    start=(i == 0), stop=(i == 10),
                )
            nc.vector.tensor_copy(warm_out[:], pswarm[:])

            # -------- wkv + x^T loads, interleaved per dmodel chunk ---------
            # The first K|V matmul needs only (wkv chunk 0, xt chunk 0 of
            # token block 0), so those bytes go on the queues first and the
            # PE starts ~2us after the DMA engines spin up.  wq/wo are only
            # needed in the tail phase, so they ride after all of x^T.
            wkv_sb = pers.tile([128, DM_C, 2 * QD], MM_DT, tag="wkv")
            wq_sb = pers.tile([128, DM_C, QD], MM_DT, tag="wq")
            xt_sb = pers.tile([128, DM_C, L], MM_DT, tag="xt")
            bq_sb = pers.tile([128, QD_C], F32, tag="bq")
            bk_sb = pers.tile([1, QD], F32, tag="bk1")
            bv_sb = pers.tile([1, QD], F32, tag="bv1")
            bk_bc = pers.tile([128, QD], F32, tag="bkbc")
            bv_bc = pers.tile([128, QD], F32, tag="bvbc")
            fsl0 = slice(0, 512)
            for dc in range(DM_C):
                rsl = slice(dc * 128, (dc + 1) * 128)
                nc.sync.dma_start(wkv_sb[:, dc, :], wkv[rsl, :])
                nc.sync.dma_start(xt_sb[:, dc, fsl0], xT[rsl, fsl0])
                if dc == 1:
                    # biases ride after the first two chunk pairs; their
                    # consumers (K/V copy-backs) fire much later
                    nc.sync.dma_start(bq_sb[:], bqt.ap())
                    nc.sync.dma_start(bk_sb[:], bkd.ap())
                    nc.sync.dma_start(bv_sb[:], bvd.ap())
                    nc.gpsimd.partition_broadcast(bk_bc[:], bk_sb[:])
                    nc.gpsimd.partition_broadcast(bv_bc[:], bv_sb[:])
            for tf in range(1, T_F):
                fsl = slice(tf * 512, (tf + 1) * 512)
                for dc in range(DM_C):
                    nc.sync.dma_start(
                        xt_sb[:, dc, fsl], xT[dc * 128:(dc + 1) * 128, fsl]
                    )
            for dc in range(DM_C):
                rsl = slice(dc * 128, (dc + 1) * 128)
                nc.sync.dma_start(wq_sb[:, dc, :], wq[rsl, :])
            wo_sb = pers.tile([128, QD_C, DM], MM_DT, tag="wo")
            nc.sync.dma_start(
                wo_sb[:], wo.ap().rearrange("(c p) n -> p c n", p=128)
            )

            qT_sb = pers.tile([128, QD_C, L], MM_DT, tag="qT")
            k_sb = pers.tile([128, T_N, QD], MM_DT, tag="k")
            v_sb = pers.tile([128, T_N, QD], MM_DT, tag="v")
            mt_bd = pers.tile([128, QD_C, 128], MM_DT, tag="mt")
            wt_sb = pers.tile([128, QD_C, DM], MM_DT, tag="wt")

            nc.vector.memzero(mt_bd[:])

            # -------- P2: K|V projections, chasing the x^T loads ------------
            for tf in range(T_F):
                for tl in range(4):
                    tn = 4 * tf + tl
                    tsl = slice(tn * 128, (tn + 1) * 128)
                    pskv = ps.tile([128, 512], F32, tag="ps", name="pskv")
                    for dc in range(DM_C):
                        nc.tensor.matmul(
                            pskv[:],
                            lhsT=xt_sb[:, dc, tsl],
                            rhs=wkv_sb[:, dc, :],
                            start=(dc == 0),
                            stop=(dc == DM_C - 1),
                        )
                    nc.vector.tensor_add(k_sb[:, tn, :], pskv[:, 0:QD], bk_bc[:])
                    nc.vector.tensor_add(v_sb[:, tn, :], pskv[:, QD:2 * QD], bv_bc[:])

            # ---------------- P3: MT = V^T K per head-pair ------------------
            # Accumulation chases P2: MM #tn only needs K/V chunk tn.
            for hp in range(QD_C):
                psm = ps.tile([128, 256], F32, tag="ps", name="psm")
                for tn in range(T_N):
                    nc.tensor.matmul(
                        psm[:],
                        lhsT=v_sb[:, tn, hp * 128:(hp + 1) * 128],
                        rhs=k_sb[:, tn, :],
                        start=(tn == 0),
                        stop=(tn == T_N - 1),
                    )
                # mt_bd[:, hp] = blockdiag(M_h0^T, M_h1^T) * SCALE
                for j in range(2):
                    sl = slice(64 * j, 64 * j + 64)
                    nc.vector.tensor_scalar_mul(
                        mt_bd[sl, hp, sl],
                        psm[sl, 128 * hp + 64 * j:128 * hp + 64 * j + 64],
                        SCALE,
                    )

            # ---------------- P4: W~ = mt_bd^T @ Wo_pair --------------------
            for hp in range(QD_C):
                for oc in range(OUT_F):
                    psw = ps.tile([128, 512], F32, tag="ps", name="psw")
                    nc.tensor.matmul(
                        psw[:],
                        lhsT=mt_bd[:, hp, :],
                        rhs=wo_sb[:, hp, oc * 512:(oc + 1) * 512],
                        start=True,
                        stop=True,
                    )
                    nc.vector.tensor_copy(
                        wt_sb[:, hp, oc * 512:(oc + 1) * 512], psw[:]
                    )

            # ------- tail: P1 + P5 interleaved per token block -> DRAM -----
            # Everything after P4 overlaps the output DMA.  Copy-backs are
            # split between DVE and ACT so neither engine paces the tail.
            for tf in range(T_F):
                fsl = slice(tf * 512, (tf + 1) * 512)
                for qc in range(QD_C):
                    psq = ps.tile([128, 512], F32, tag="ps", name="psq")
                    for dc in range(DM_C):
                        nc.tensor.matmul(
                            psq[:],
                            lhsT=wq_sb[:, dc, qc * 128:(qc + 1) * 128],
                            rhs=xt_sb[:, dc, fsl],
                            start=(dc == 0),
                            stop=(dc == DM_C - 1),
                        )
                    nc.vector.tensor_scalar_add(
                        qT_sb[:, qc, fsl], psq[:], bq_sb[:, qc:qc + 1]
                    )

                for tl in range(4):
                    tn = 4 * tf + tl
                    tsl = slice(tn * 128, (tn + 1) * 128)
                    p_tile = pstage.tile([128, DM], F32, tag="pstage",
                                         name="p_tile")
                    for oc in range(OUT_F):
                        psp = ps.tile([128, 512], F32, tag="ps", name="psp")
                        for qc in range(QD_C):
                            nc.tensor.matmul(
                                psp[:],
                                lhsT=qT_sb[:, qc, tsl],
                                rhs=wt_sb[:, qc, oc * 512:(oc + 1) * 512],
                                start=(qc == 0),
                                stop=(qc == QD_C - 1),
                            )
                        osl = slice(oc * 512, (oc + 1) * 512)
                        if oc == 0:
                            nc.vector.tensor_copy(p_tile[:, osl], psp[:])
                        else:
                            nc.scalar.copy(p_tile[:, osl], psp[:])
                        nc.sync.dma_start(pout[tsl, osl], p_tile[:, osl])

    nc.compile()
    return nc


_PROGRAM = None


def _get_program():
    global _PROGRAM
    if _PROGRAM is None:
        _PROGRAM = build_program()
    return _PROGRAM


def _tf32_round(a):
    """Round float32 array to tf32 (10-bit mantissa), round-to-nearest-even."""
    if not USE_F32R:
        return np.ascontiguousarray(a, np.float32)
    u = np.ascontiguousarray(a, np.float32).view(np.uint32)
    r = (u + np.uint32(0xFFF) + ((u >> np.uint32(13)) & np.uint32(1))) & np.uint32(
        0xFFFFE000
    )
    return r.view(np.float32)


def kernel(x, Wq, bq, Wk, bk, Wv, bv, Wo, bo, _trace=False, _trace_kwargs=None):
    x = np.asarray(x, np.float32)
    Wq, bq = np.asarray(Wq, np.float32), np.asarray(bq, np.float32)
    Wk, bk = np.asarray(Wk, np.float32), np.asarray(bk, np.float32)
    Wv, bv = np.asarray(Wv, np.float32), np.asarray(bv, np.float32)
    Wo, bo = np.asarray(Wo, np.float32), np.asarray(bo, np.float32)

    nc = _get_program()

    xT = [_tf32_round(x[b].T) for b in range(B)]
    in_maps = []
    for c in range(NCORES):
        b, g = divmod(c, NCORES // B)
        sl = slice(g * QD, (g + 1) * QD)
        in_maps.append({
            "xT": xT[b],
            "wq": _tf32_round(Wq[:, sl]),
            "wkv": _tf32_round(np.concatenate([Wk[:, sl], Wv[:, sl]], axis=1)),
            "wo": _tf32_round(Wo[sl, :]),
            "bqt": np.ascontiguousarray(bq[sl].reshape(QD_C, 128).T),
            "bkd": np.ascontiguousarray(bk[sl].reshape(1, QD)),
            "bvd": np.ascontiguousarray(bv[sl].reshape(1, QD)),
        })

    kw = {}
    if _trace:
        kw = dict(trace=True, trace_cores=list(range(NCORES)),
                  **(_trace_kwargs or {}))
    res = run_bass_kernel_spmd(nc, in_maps, list(range(NCORES)), **kw)

    out = np.empty((B, L, DM), np.float32)
    gpb = NCORES // B
    for b in range(B):
        acc = res.results[gpb * b]["pout"].astype(np.float32)
        for i in range(1, gpb):
            acc = acc + res.results[gpb * b + i]["pout"]
        out[b] = acc + bo
    kernel.last_results = res
    return out
